# revision 21
# baseline (speedup 1.0000x reference)
"""ANI-1x AEV (radial + angular symmetry functions) on 8 Trainium2 NeuronCores.

Sharding: data-parallel over AEV centers. Core c computes rows [32c, 32c+32)
of the [256, 48] output; coordinate/charge arrays are replicated to every
core (plus a pre-sliced `centers` tensor so the SPMD graph knows its shard).

Per-core pipeline (all arithmetic on device):
  1. dense pair pass for the radial AEV at layout [128=(jgrp,center), 64 j]
  2. angular neighbor compaction: d^2 mask -> cumsum scan -> slot ids at
     [32, 256], PE-transpose, one-hot selection matrix, PE matmul-gather
     of (x,y,z,q) for up to J=24 neighbors per center
  3. triple stage at [128=(jgrp,center), 6*24 (j,k) pairs] using
     cos(theta - shf) = c*cos(shf) + sqrt(1-c^2)*sin(shf)  (no arccos)
     and t^32 = exp(32 ln t); fused multiply+reduce for the (a,z) sums.

Scalar-engine (ACT) calls are emitted grouped by LUT table-set
(sqrt -> sin -> square -> exp -> ln -> exp) — each set switch costs ~2.7us.
ACT Sin is only accurate on [0, pi]; cutoffs use fc = sin^2(pi/2 - pi*d/2Rc).
"""

import math

import numpy as np

from concourse import bass, mybir, bacc
import concourse.tile as tile
from concourse.bass_utils import run_bass_kernel_spmd
from concourse.masks import make_identity

F32 = mybir.dt.float32
I32 = mybir.dt.int32
ALU = mybir.AluOpType
ACTF = mybir.ActivationFunctionType

# problem constants (ANI-1x rHCNO-5.2R_16-3.5A_a4-8)
N = 256          # atoms
C = 32           # centers per core
P = 128          # partitions
JG = 4           # j groups per center (C*JG == P)
JS = 6           # j slots per group
J = JG * JS      # 24 angular neighbor slots (data max is 22)
JR = N // JG     # 64 j per group for the dense radial pass
M = 16           # radial shifts
A = 4            # angular radial shifts
Z = 8            # angle shifts
JK = JS * J
RCR = 5.2
RCA = 3.5
ETA_R = 16.0
ETA_A = 8.0
SQ095 = math.sqrt(0.95)
PI = math.pi


def _bc(ap, axis, n):
    """Insert a broadcast (step-0) dim of size n at `axis`."""
    shape = list(ap.shape)
    shape.insert(axis, n)
    return ap.unsqueeze(axis).to_broadcast(shape)


def build_nc(core_id: int, debug: bool = False):
    del core_id  # same SPMD graph on every core; shard arrives via `centers`
    nc = bacc.Bacc("TRN2", target_bir_lowering=False, debug=False)
    coords = nc.declare_dram_parameter("coordinates", [N, 3], F32, isOutput=False)
    charges = nc.declare_dram_parameter("charges", [N], F32, isOutput=False)
    centers = nc.declare_dram_parameter("centers", [C, 3], F32, isOutput=False)
    out_ext = nc.declare_dram_parameter("out", [C, M + A * Z], F32, isOutput=True)
    dbg = {}
    if debug:
        for nm, shp in [("slotm", [C, N]), ("p48", [P, 48]),
                        ("kvjv", [P, 30 * 4])]:
            dbg[nm] = nc.declare_dram_parameter(f"dbg_{nm}", shp, F32, isOutput=True)

    with tile.TileContext(nc) as tc:
        with tc.tile_pool(name="sb", bufs=1) as sb, \
             tc.tile_pool(name="ps", bufs=1, space="PSUM") as ps, \
             tc.tile_pool(name="dr", bufs=1, space="DRAM") as dr:
            _build_body(nc, tc, sb, ps, dr, coords, charges, centers, out_ext, dbg)
    nc.compile()
    return nc


def _build_body(nc, tc, sb, ps, dr, coords, charges, centers, out_ext, dbg):
    v = nc.vector
    g = nc.gpsimd
    s = nc.scalar
    dma = nc.sync.dma_start

    # ============ deferred constants (cast-free) ============
    halfpi = sb.tile([P, 1], F32, name="halfpi")
    g.memset(halfpi[:], PI / 2.0)
    one_col = sb.tile([P, 1], F32, name="one_col")
    g.memset(one_col[:], 1.0)
    iif = sb.tile([P, C], F32, name="iif")
    g.iota(iif[:], pattern=[[1, C]], base=0, channel_multiplier=0,
           allow_small_or_imprecise_dtypes=True)
    shfr = sb.tile([P, M], F32, name="shfr")
    v.tensor_scalar(shfr[:], iif[:, :M], 0.26875, 0.9, ALU.mult, ALU.add)
    shfa = sb.tile([P, A], F32, name="shfa")
    v.tensor_scalar(shfa[:], iif[:, :A], 0.65, 0.9, ALU.mult, ALU.add)
    thz = sb.tile([P, Z], F32, name="thz")
    v.tensor_scalar(thz[:], iif[:, :Z], PI / 8.0, PI / 16.0, ALU.mult, ALU.add)
    pcmodf = sb.tile([P, 1], F32, name="pcmodf")  # p % 32 per partition
    for gi in range(JG):
        g.iota(pcmodf[gi * C:(gi + 1) * C, :], pattern=[[0, 1]], base=0,
               channel_multiplier=1, allow_small_or_imprecise_dtypes=True)
    selfi = sb.tile([P, C], F32, name="selfi")  # [p, c] = (p % 32 == c)
    v.tensor_scalar(selfi[:], iif[:], pcmodf[:, 0:1], None, ALU.is_equal)
    jbasef = sb.tile([P, 1], F32, name="jbasef")  # 6 * (p // 32)
    for gi in range(JG):
        g.memset(jbasef[gi * C:(gi + 1) * C, :], float(JS * gi))
    slotj = sb.tile([P, JS], F32, name="slotj")  # absolute j-slot per partition
    v.tensor_scalar(slotj[:], iif[:, :JS], jbasef[:, 0:1], None, ALU.add)

    # ============ DVE op-table warmups (run during input DMA wait) ============
    wsrc = sb.tile([P, 2], F32, name="wsrc")
    g.memset(wsrc[:], 1.0)
    wdst = sb.tile([P, 2], F32, name="wdst")
    wacc = sb.tile([P, 1], F32, name="wacc")
    v.tensor_mul(wdst[:], wsrc[:], wsrc[:])
    v.tensor_add(wdst[:], wsrc[:], wsrc[:])
    v.tensor_sub(wdst[:], wsrc[:], wsrc[:])
    v.tensor_copy(wdst[:], wsrc[:])
    v.tensor_scalar(wdst[:], wsrc[:], 1.0, None, ALU.subtract)
    v.tensor_scalar(wdst[:], wsrc[:], 1.0, 0.5, ALU.mult, ALU.add)
    v.tensor_scalar(wdst[:], wsrc[:], 1.0, None, ALU.is_gt)
    v.scalar_tensor_tensor(wdst[:], wsrc[:], 1.0, wsrc[:], ALU.is_lt, ALU.mult)
    v.scalar_tensor_tensor(wdst[:], wsrc[:], 1.0, wsrc[:], ALU.mult, ALU.mult,
                           accum_out=wacc[:])
    v.scalar_tensor_tensor(wdst[:], wsrc[:], 1.0, wsrc[:], ALU.add, ALU.add)
    v.scalar_tensor_tensor(wdst[:], wsrc[:], 1.0, wsrc[:], ALU.mult, ALU.add)
    v.tensor_tensor(wdst[:], wsrc[:], wsrc[:], ALU.is_equal)
    v.tensor_tensor(wdst[:], wsrc[:], wsrc[:], ALU.not_equal)
    v.tensor_tensor_scan(wdst[:], wsrc[:], wsrc[:], 0.0, ALU.add, ALU.bypass)
    v.tensor_reduce(wacc[:], wsrc[:], mybir.AxisListType.X, ALU.add)
    v.reciprocal(wdst[:], wsrc[:])

    # ============ gather-path constants (high priority) ============
    scf = sb.tile([P, C * J], F32, name="scf")  # Sel grid: value s at (c, s)
    g.iota(scf[:], pattern=[[0, C], [1, J]], base=0, channel_multiplier=0,
           allow_small_or_imprecise_dtypes=True)
    ident = sb.tile([C, C], F32, name="ident")
    make_identity(nc, ident[:])

    # ============ input loads ============
    # partition order for [P]-tiles is p = g*C + c  (jgroup-major)
    cen32 = sb.tile([C, 3], F32, name="cen32")
    dma(out=cen32[:], in_=centers[:])
    xyzj = sb.tile([C, 3 * N], F32, name="xyzj")  # [c, (j, d)]
    dma(out=xyzj[:], in_=_bc(coords[:].rearrange("j d -> (j d)"), 0, C))
    cen128 = sb.tile([P, 3], F32, name="cen128")
    for gi in range(JG):
        nc.scalar.dma_start(out=cen128[gi * C:(gi + 1) * C, :], in_=centers[:])
    xyzr = sb.tile([P, 3 * JR], F32, name="xyzr")  # [(g,c), (j, d)]
    for gi in range(JG):
        nc.scalar.dma_start(
            out=xyzr[gi * C:(gi + 1) * C, :],
            in_=_bc(coords[gi * JR:(gi + 1) * JR, :].rearrange("j d -> (j d)"), 0, C))
    qr = sb.tile([P, JR], F32, name="qr")
    for gi in range(JG):
        nc.scalar.dma_start(
            out=qr[gi * C:(gi + 1) * C, :],
            in_=_bc(charges[gi * JR:(gi + 1) * JR], 0, C))
    dat0 = sb.tile([P, 4], F32, name="dat0")
    dma(out=dat0[:, 0:3], in_=coords[0:P, :])
    dma(out=dat0[:, 3:4], in_=charges[0:P].unsqueeze(1))
    dat1 = sb.tile([P, 4], F32, name="dat1")
    dma(out=dat1[:, 0:3], in_=coords[P:N, :])
    dma(out=dat1[:, 3:4], in_=charges[P:N].unsqueeze(1))

    # ============ angular mask + slot scan at [32, 256] ============
    xj = xyzj[:].rearrange("c (j d) -> c j d", d=3)
    dxm = sb.tile([C, N], F32, name="dxm")
    dym = sb.tile([C, N], F32, name="dym")
    dzm = sb.tile([C, N], F32, name="dzm")
    v.tensor_scalar(dxm[:], xj[:, :, 0], cen32[:, 0:1], None, ALU.subtract)
    v.tensor_scalar(dym[:], xj[:, :, 1], cen32[:, 1:2], None, ALU.subtract)
    v.tensor_scalar(dzm[:], xj[:, :, 2], cen32[:, 2:3], None, ALU.subtract)
    dsqm = sb.tile([C, N], F32, name="dsqm")
    tmpm = sb.tile([C, N], F32, name="tmpm")
    v.tensor_mul(dsqm[:], dxm[:], dxm[:])
    v.tensor_mul(tmpm[:], dym[:], dym[:])
    v.tensor_add(dsqm[:], dsqm[:], tmpm[:])
    v.tensor_mul(tmpm[:], dzm[:], dzm[:])
    v.tensor_add(dsqm[:], dsqm[:], tmpm[:])
    m2 = sb.tile([C, N], F32, name="m2")
    v.tensor_scalar(m2[:], dsqm[:], 0.0, None, ALU.is_gt)
    mask = sb.tile([C, N], F32, name="mask")  # (dsq < Rca^2) * (dsq > 0)
    v.scalar_tensor_tensor(mask[:], dsqm[:], RCA * RCA, m2[:], ALU.is_lt, ALU.mult)
    incl = sb.tile([C, N], F32, name="incl")
    v.tensor_tensor_scan(incl[:], mask[:], mask[:], 0.0, ALU.add, ALU.bypass)
    slot = sb.tile([C, N], F32, name="slot")
    v.tensor_sub(slot[:], incl[:], mask[:])
    slotm = sb.tile([C, N], F32, name="slotm")  # masked-out -> slot + 999
    v.scalar_tensor_tensor(slotm[:], mask[:], -999.0, slot[:], ALU.mult, ALU.add)
    slotm2 = sb.tile([C, N], F32, name="slotm2")
    v.tensor_scalar_add(slotm2[:], slotm[:], 999.0)
    if "slotm" in dbg:
        dma(out=dbg["slotm"][:], in_=slotm2[:])

    # ============ transpose -> one-hot Sel -> matmul gather ============
    psg = ps.tile([C * 3, C], F32, name="psg")  # [96=(ci,s), 32=(b,q)]
    sels = []
    for jc in range(2):
        pt = ps.tile([P, C], F32, name=f"pt{jc}")
        nc.tensor.transpose(pt[:], slotm2[:, jc * P:(jc + 1) * P], ident[:])
        st = sb.tile([P, C], F32, name=f"st{jc}")
        v.tensor_copy(st[:], pt[:])
        sel = sb.tile([P, C * J], F32, name=f"sel{jc}")
        v.tensor_tensor(sel[:].rearrange("p (c ss) -> p c ss", c=C),
                        _bc(st[:], 2, J),
                        scf[:].rearrange("p (c ss) -> p c ss", c=C),
                        ALU.is_equal)
        sels.append(sel)
    for b in range(8):
        for jc in range(2):
            nc.tensor.matmul(
                psg[:, b * 4:(b + 1) * 4],
                lhsT=sels[jc][:, b * (4 * J):(b + 1) * (4 * J)],
                rhs=(dat0 if jc == 0 else dat1)[:],
                start=(jc == 0), stop=(jc == 1))
    # per-block copy + spill, pipelined against the remaining matmuls
    nbraw = sb.tile([C * 3, C], F32, name="nbraw")
    u0 = dr.tile([8, 4 * J * 4], F32, name="u0")
    spill_eng = [nc.sync, nc.scalar, nc.gpsimd]
    for b in range(8):
        v.tensor_copy(nbraw[:, b * 4:(b + 1) * 4], psg[:, b * 4:(b + 1) * 4])
        spill_eng[b % 3].dma_start(out=u0[b:b + 1, :], in_=nbraw[:, b * 4:(b + 1) * 4])

    # combined neighbor tile: cols 0..24 = k slots, 24..30 = j slots; (slot, q)
    kvjv = sb.tile([P, 30 * 4], F32, name="kvjv")
    src_kv = u0[:].rearrange("b (ci k q) -> (b ci) (k q)", ci=4, k=J)
    kvv = kvjv[:].rearrange("p (t q) -> p t q", q=4)
    for gi in range(JG):
        dma(out=kvv[gi * C:(gi + 1) * C, 0:J, :], in_=src_kv)
    for gi in range(JG):
        v.tensor_copy(
            kvjv[gi * C:(gi + 1) * C, J * 4:30 * 4],
            kvjv[gi * C:(gi + 1) * C, gi * JS * 4:(gi + 1) * JS * 4])
    if "kvjv" in dbg:
        dma(out=dbg["kvjv"][:], in_=kvjv[:])

    # ============ per-pair quantities on [P, 30] ============
    W30 = 30
    rawx = kvv[:, :, 0]
    rawy = kvv[:, :, 1]
    rawz = kvv[:, :, 2]
    rawq = kvv[:, :, 3]
    dx = sb.tile([P, W30], F32, name="dx")
    dy = sb.tile([P, W30], F32, name="dy")
    dz = sb.tile([P, W30], F32, name="dz")
    v.tensor_scalar(dx[:], rawx, cen128[:, 0:1], None, ALU.subtract)
    v.tensor_scalar(dy[:], rawy, cen128[:, 1:2], None, ALU.subtract)
    v.tensor_scalar(dz[:], rawz, cen128[:, 2:3], None, ALU.subtract)
    dsq = sb.tile([P, W30], F32, name="dsq")
    tmp0 = sb.tile([P, W30], F32, name="tmp0")
    v.tensor_mul(dsq[:], dx[:], dx[:])
    v.tensor_mul(tmp0[:], dy[:], dy[:])
    v.tensor_add(dsq[:], dsq[:], tmp0[:])
    v.tensor_mul(tmp0[:], dz[:], dz[:])
    v.tensor_add(dsq[:], dsq[:], tmp0[:])

    # --- radial pair pass (dense [P, 64]) — subs on DVE, squares on gpsimd
    xr = xyzr[:].rearrange("p (j d) -> p j d", d=3)
    dxr = sb.tile([P, JR], F32, name="dxr")
    dyr = sb.tile([P, JR], F32, name="dyr")
    dzr = sb.tile([P, JR], F32, name="dzr")
    v.tensor_scalar(dxr[:], xr[:, :, 0], cen128[:, 0:1], None, ALU.subtract)
    v.tensor_scalar(dyr[:], xr[:, :, 1], cen128[:, 1:2], None, ALU.subtract)
    v.tensor_scalar(dzr[:], xr[:, :, 2], cen128[:, 2:3], None, ALU.subtract)
    dsqr = sb.tile([P, JR], F32, name="dsqr")
    tmpr = sb.tile([P, JR], F32, name="tmpr")
    v.tensor_mul(dsqr[:], dxr[:], dxr[:])
    v.tensor_mul(tmpr[:], dyr[:], dyr[:])
    v.tensor_add(dsqr[:], dsqr[:], tmpr[:])
    v.tensor_mul(tmpr[:], dzr[:], dzr[:])
    v.tensor_add(dsqr[:], dsqr[:], tmpr[:])

    # ============ ACT group 1: Sqrt (angular first; radial gated on it) ====
    d = sb.tile([P, W30], F32, name="d")
    s.activation(d[:], dsq[:], ACTF.Sqrt)
    # gate the radial transcendental chain behind the angular one so the
    # scheduler can't hoist it ahead and double every ACT table load
    dsqr2 = sb.tile([P, JR], F32, name="dsqr2")
    v.scalar_tensor_tensor(dsqr2[:], dsqr[:], d[:, 0:1], dsqr[:],
                           ALU.bypass, ALU.bypass)
    ddr = sb.tile([P, JR], F32, name="ddr")
    s.activation(ddr[:], dsqr2[:], ACTF.Sqrt)

    # pair chains (DVE)
    rinv = sb.tile([P, W30], F32, name="rinv")
    v.reciprocal(rinv[:], d[:])
    us = sb.tile([P, W30], F32, name="us")
    v.tensor_scalar_mul(us[:], rinv[:], SQ095)
    ux = sb.tile([P, W30], F32, name="ux")
    uy = sb.tile([P, W30], F32, name="uy")
    uz = sb.tile([P, W30], F32, name="uz")
    v.tensor_mul(ux[:], dx[:], us[:])
    v.tensor_mul(uy[:], dy[:], us[:])
    v.tensor_mul(uz[:], dz[:], us[:])
    hd = sb.tile([P, W30], F32, name="hd")
    v.tensor_scalar_mul(hd[:], d[:], 0.5)

    # triple geometry (cc/csq feed the sth Sqrt, still ACT group 1)
    def kk(t):
        return t[:, 0:J]

    def jj(t):
        return t[:, J:W30]

    def obc(apj, apk):
        return _bc(apj, 2, J), _bc(apk, 1, JS)

    cc = sb.tile([P, JK], F32, name="cc")
    tmp3 = sb.tile([P, JK], F32, name="tmp3")
    aj, ak = obc(jj(ux[:]), kk(ux[:]))
    v.tensor_tensor(cc[:].rearrange("p (j k) -> p j k", j=JS), aj, ak, ALU.mult)
    aj, ak = obc(jj(uy[:]), kk(uy[:]))
    v.tensor_tensor(tmp3[:].rearrange("p (j k) -> p j k", j=JS), aj, ak, ALU.mult)
    v.tensor_add(cc[:], cc[:], tmp3[:])
    aj, ak = obc(jj(uz[:]), kk(uz[:]))
    v.tensor_tensor(tmp3[:].rearrange("p (j k) -> p j k", j=JS), aj, ak, ALU.mult)
    v.tensor_add(cc[:], cc[:], tmp3[:])
    csq = sb.tile([P, JK], F32, name="csq")
    v.tensor_mul(csq[:], cc[:], cc[:])
    sth = sb.tile([P, JK], F32, name="sth")
    s.activation(sth[:], csq[:], ACTF.Sqrt, bias=one_col[:], scale=-1.0)

    # ============ ACT group 2: Sin ============
    azh = sb.tile([P, Z], F32, name="azh")
    s.activation(azh[:], thz[:], ACTF.Sin, scale=0.5)   # sin(thz/2)
    bz = sb.tile([P, Z], F32, name="bz")
    s.activation(bz[:], thz[:], ACTF.Sin)               # sin(thz)
    snr = sb.tile([P, JR], F32, name="snr")
    s.activation(snr[:], ddr[:], ACTF.Sin, bias=halfpi[:], scale=-PI / (2 * RCR))
    sn = sb.tile([P, W30], F32, name="sn")
    s.activation(sn[:], d[:], ACTF.Sin, bias=halfpi[:], scale=-PI / (2 * RCA))

    # angular-shift constants from azh/bz (DVE)
    azh2 = sb.tile([P, Z], F32, name="azh2")
    v.tensor_mul(azh2[:], azh[:], azh[:])
    az2 = sb.tile([P, Z], F32, name="az2")
    v.tensor_scalar(az2[:], azh2[:], -1.0, 0.5, ALU.mult, ALU.add)  # 0.5 cos
    bz2 = sb.tile([P, Z], F32, name="bz2")
    v.tensor_scalar_mul(bz2[:], bz[:], 0.5)                          # 0.5 sin

    # angular fc * q with cutoff mask folded (DVE)
    fc = sb.tile([P, W30], F32, name="fc")
    v.tensor_mul(fc[:], sn[:], sn[:])
    fcm = sb.tile([P, W30], F32, name="fcm")
    v.scalar_tensor_tensor(fcm[:], d[:], RCA, fc[:], ALU.is_lt, ALU.mult)
    fcq = sb.tile([P, W30], F32, name="fcq")
    v.tensor_mul(fcq[:], fcm[:], rawq)

    # radial fc chain (gpsimd square, DVE fused masks)
    fcr = sb.tile([P, JR], F32, name="fcr")
    v.tensor_mul(fcr[:], snr[:], snr[:])
    fcr2 = sb.tile([P, JR], F32, name="fcr2")
    v.scalar_tensor_tensor(fcr2[:], ddr[:], RCR, fcr[:], ALU.is_lt, ALU.mult)
    fcr3 = sb.tile([P, JR], F32, name="fcr3")
    v.scalar_tensor_tensor(fcr3[:], dsqr[:], 0.0, fcr2[:], ALU.is_gt, ALU.mult)
    fcqr = sb.tile([P, JR], F32, name="fcqr")
    v.scalar_tensor_tensor(fcqr[:], fcr3[:], 0.25, qr[:], ALU.mult, ALU.mult)

    # triple weights / davg (DVE)
    davg = sb.tile([P, JK], F32, name="davg")
    aj, ak = obc(jj(hd[:]), kk(hd[:]))
    v.tensor_tensor(davg[:].rearrange("p (j k) -> p j k", j=JS), aj, ak, ALU.add)
    ww = sb.tile([P, JK], F32, name="ww")
    aj, ak = obc(jj(fcq[:]), kk(fcq[:]))
    v.tensor_tensor(ww[:].rearrange("p (j k) -> p j k", j=JS), aj, ak, ALU.mult)
    eyem = sb.tile([P, JK], F32, name="eyem")  # 1 where slot_j != slot_k
    v.tensor_tensor(eyem[:].rearrange("p (j k) -> p j k", j=JS),
                    _bc(slotj[:], 2, J), _bc(iif[:, :J], 1, JS), ALU.not_equal)
    wwm = sb.tile([P, JK], F32, name="wwm")
    v.tensor_mul(wwm[:], ww[:], eyem[:])

    # rad_a argument (DVE sub), radial m-grid (gpsimd sub)
    dsh = sb.tile([P, A * JK], F32, name="dsh")
    v.tensor_tensor(dsh[:].rearrange("p (a f) -> p a f", a=A),
                    _bc(davg[:], 1, A), _bc(shfa[:], 2, JK), ALU.subtract)
    dmr = sb.tile([P, M * JR], F32, name="dmr")
    v.tensor_tensor(dmr[:].rearrange("p (m j) -> p m j", m=M),
                    _bc(ddr[:], 1, M), _bc(shfr[:], 2, JR), ALU.subtract)

    # ============ ACT group 3: Square ============
    dshsq = sb.tile([P, A * JK], F32, name="dshsq")
    s.activation(dshsq[:], dsh[:], ACTF.Square)
    dmsq = sb.tile([P, M * JR], F32, name="dmsq")
    s.activation(dmsq[:], dmr[:], ACTF.Square)

    # t = 0.5 + az*c + bz*s in 2 z-chunks (DVE)
    ZC = Z // 2
    tts = []
    for zc in range(2):
        zs = slice(zc * ZC, (zc + 1) * ZC)
        p1 = sb.tile([P, ZC * JK], F32, name=f"p1_{zc}")
        v.tensor_tensor(p1[:].rearrange("p (z f) -> p z f", z=ZC),
                        _bc(cc[:], 1, ZC), _bc(az2[:, zs], 2, JK), ALU.mult)
        p2 = sb.tile([P, ZC * JK], F32, name=f"p2_{zc}")
        v.tensor_tensor(p2[:].rearrange("p (z f) -> p z f", z=ZC),
                        _bc(sth[:], 1, ZC), _bc(bz2[:, zs], 2, JK), ALU.mult)
        tt0 = sb.tile([P, ZC * JK], F32, name=f"tt0_{zc}")
        v.scalar_tensor_tensor(tt0[:], p1[:], 0.5, p2[:], ALU.add, ALU.add)
        tts.append(tt0)

    # ============ ACT group 4: Exp ============
    rada = sb.tile([P, A * JK], F32, name="rada")
    s.activation(rada[:], dshsq[:], ACTF.Exp, scale=-ETA_A)
    emr = sb.tile([P, M * JR], F32, name="emr")
    s.activation(emr[:], dmsq[:], ACTF.Exp, scale=-ETA_R)

    # ============ ACT groups 5+6: Ln then Exp(32x) ============
    tlns = []
    for zc in range(2):
        tln = sb.tile([P, ZC * JK], F32, name=f"tln_{zc}")
        s.activation(tln[:], tts[zc][:], ACTF.Ln)
        tlns.append(tln)
    t32s = []
    for zc in range(2):
        t32 = sb.tile([P, ZC * JK], F32, name=f"t32_{zc}")
        s.activation(t32[:], tlns[zc][:], ACTF.Exp, scale=32.0)
        t32s.append(t32)

    # rw = rad_a * w (DVE)
    rw = sb.tile([P, A * JK], F32, name="rw")
    v.tensor_tensor(rw[:].rearrange("p (a f) -> p a f", a=A),
                    rada[:].rearrange("p (a f) -> p a f", a=A),
                    _bc(wwm[:], 1, A), ALU.mult)

    # radial features: product on gpsimd, per-m reduce on DVE
    p48 = sb.tile([P, 48], F32, name="p48")
    prr = sb.tile([P, M * JR], F32, name="prr")
    v.tensor_tensor(prr[:].rearrange("p (m j) -> p m j", m=M),
                    emr[:].rearrange("p (m j) -> p m j", m=M),
                    _bc(fcqr[:], 1, M), ALU.mult)
    v.tensor_reduce(p48[:, 0:M], prr[:].rearrange("p (m j) -> p m j", m=M),
                    mybir.AxisListType.X, ALU.add)

    # fused multiply + free reduce for each (a, z) (DVE)
    outza = sb.tile([P, A * Z * JK], F32, name="outza")
    rwv = rw[:].rearrange("p (a f) -> p a f", a=A)
    ozv = outza[:].rearrange("p (az f) -> p az f", az=A * Z)
    for zc in range(2):
        t32v = t32s[zc][:].rearrange("p (z f) -> p z f", z=ZC)
        for a in range(A):
            for zz in range(ZC):
                z = zc * ZC + zz
                col = M + a * Z + z
                v.scalar_tensor_tensor(
                    ozv[:, a * Z + z, :], t32v[:, zz, :], 1.0, rwv[:, a, :],
                    ALU.mult, ALU.mult, accum_out=p48[:, col:col + 1])

    if "p48" in dbg:
        dma(out=dbg["p48"][:], in_=p48[:])

    # ============ cross-jgroup reduce via PE + store ============
    pso = ps.tile([C, 48], F32, name="pso")
    nc.tensor.matmul(pso[:], lhsT=selfi[:], rhs=p48[:], start=True, stop=True)
    outt = sb.tile([C, 48], F32, name="outt")
    v.tensor_copy(outt[:], pso[:])
    dma(out=out_ext[:], in_=outt[:])


_CACHE = {}


def _get_nc(debug=False):
    key = bool(debug)
    if key not in _CACHE:
        _CACHE[key] = build_nc(0, debug=debug)
    return _CACHE[key]


def kernel(coordinates: np.ndarray, charges: np.ndarray, _debug=False):
    coordinates = np.ascontiguousarray(coordinates, dtype=np.float32)
    charges = np.ascontiguousarray(charges, dtype=np.float32)
    assert coordinates.shape == (N, 3) and charges.shape == (N,)
    nc = _get_nc(debug=_debug)
    in_maps = [
        {"coordinates": coordinates, "charges": charges,
         "centers": coordinates[C * i:C * (i + 1)]}
        for i in range(8)
    ]
    res = run_bass_kernel_spmd(nc, in_maps, core_ids=list(range(8)))
    out = np.concatenate([res.results[i]["out"] for i in range(8)], axis=0)
    if _debug:
        dbgs = [{k: res.results[i][k] for k in res.results[i] if k.startswith("dbg_")}
                for i in range(8)]
        return out, dbgs
    return out
